# revision 19
# baseline (speedup 1.0000x reference)
"""ConvLSTM (nn_BottomConvLSTM) Trainium2 Bass kernel.

Problem (hardcoded):
  x:       [T=12, B=2, C=64, H=128, W=128] f32
  W_gates: [512, 192, 3, 3] f32,  b_gates: [512] f32
  W_out:   [64, 128, 3, 3] f32,   b_out:   [64] f32
  out:     [T, B, 64, H, W] f32

Sharding: 8 cores = B(2) x H-slabs(4 x 32 rows). Each core computes exactly
its 32-row slab every step; the 1-row h halos needed by the 3x3 convs come
from a per-step ReduceScatter across the 4 slab cores of each batch image:
every core multiplies its top/bottom h rows by a per-core 0/1 mask laid out
in destination-rank shard order, so the RS delivers to rank s exactly the
two neighbor rows it needs (and zeros at the image edges, reproducing SAME
padding). The exchange is kicked right after the two boundary row-tiles of
a step and lands while the six interior tiles compute, so it stays off the
PE critical path.

Convs run as shifted matmuls accumulating in PSUM. Matmul inputs are bf16
(PSUM accumulates fp32): bf16 enables the PE's Fast Weight Load path --
fp32r runs the array in fp32-HIGH mode, which disables FWL and leaves
~180ns of LDWEIGHTS per matmul partially unhidden. The x channels (64) are
packed twice along partitions with a +1 row shift so taps (dy=0,dx)+
(dy=1,dx) fuse into one K=128 matmul; (2,0)+(2,1) fuse via a +1
column-shifted packed copy; (2,2) runs against zero-padded full-K weights
(a K=64 matmul occupies only row groups 2-3 and breaks LDWEIGHTS/MATMUL
overlap ~+107ns on both sides) -> 5 matmuls instead of 9 for the x taps.
h taps are 9 full K=128 matmuls. LSTM pointwise runs on ACT (sigmoid/tanh)
+ DVE (mul/add). The output conv (M=64) pairs two row-groups into the two
column tiles of the PE's 128x64 mode (T0 -> PSUM partitions 0:64, T1 ->
64:128) so both stream concurrently; pairs are grouped after the gate
tiles (each tiling-mode switch drains the PE) with the boundary pair last
to wait on the halo exchange off the PE critical path.

Measured: ~1.49 ms (baseline 2.12 ms), rel err 3.9e-3 (tol 2e-2). The PE
runs power-throttled at 81.25% util (gpio_2, ~1.95 GHz effective); the
matmul stream is within ~2% of that throttled roofline, so further gains
need fewer PE columns (only fp8-DoubleRow would do that, and its ~3-5%
error exceeds tolerance).
"""

import os
import sys

import numpy as np

T = 12
CIN = 64
HID = 128
H_FULL = 128
W = 128
NB = 2
NSLAB = 4
SLAB = H_FULL // NSLAB  # 32
WP = W + 2  # zero-padded width
BASE = 1  # first slab row inside the h buffer
HBUF = SLAB + 2  # 1 halo row above + 32 slab rows + 1 halo row below
XROWS = HBUF  # x rows staged per step (same window as h)

N_CORES = 8
LAST_EXEC_NS = None

# Matmul input dtype. bf16 enables Fast Weight Load; PSUM stays fp32.
FAST_DTYPE = os.environ.get("KERNEL_MM_DTYPE", "bfloat16")


def _import_concourse():
    try:
        import concourse.bass  # noqa: F401
        return
    except ImportError:
        pass
    for p in ("/opt/trn_rl_repo", "/root/.axon_site/_ro/trn_rl_repo"):
        if os.path.isdir(p) and p not in sys.path:
            sys.path.insert(0, p)
    import concourse.bass  # noqa: F401


def build_nc(t_steps=T, slab=SLAB):
    _import_concourse()
    import concourse.tile as tile
    from concourse import bacc, mybir

    F32 = mybir.dt.float32
    FMM = getattr(mybir.dt, FAST_DTYPE)
    AF = mybir.ActivationFunctionType

    nc = bacc.Bacc("TRN2", target_bir_lowering=False, debug=False)
    xp = nc.dram_tensor("xp", [t_steps, 128, XROWS, WP], FMM, kind="ExternalInput").ap()
    whd = nc.dram_tensor("wh", [128, 9, 512], FMM, kind="ExternalInput").ap()
    wxpd = nc.dram_tensor("wxp", [128, 3, 512], FMM, kind="ExternalInput").ap()
    wxcd = nc.dram_tensor("wxc", [128, 512], FMM, kind="ExternalInput").ap()
    wx2d = nc.dram_tensor("wx2", [128, 512], FMM, kind="ExternalInput").ap()
    wod = nc.dram_tensor("wo", [128, 9, 64], FMM, kind="ExternalInput").ap()
    bgd = nc.dram_tensor("bg", [128, 4], F32, kind="ExternalInput").ap()
    bod = nc.dram_tensor("bo", [128, 1], F32, kind="ExternalInput").ap()
    # Per-core halo-routing masks, shard-major: slot 2j = "top halo for
    # rank j" (gets my bottom row iff j == my_rank+1), slot 2j+1 = "bottom
    # halo for rank j" (gets my top row iff j == my_rank-1).
    hmaskd = nc.dram_tensor("hmask", [128, 2 * NSLAB, WP], FMM, kind="ExternalInput").ap()
    out = nc.dram_tensor("out", [t_steps, 64, slab, W], F32, kind="ExternalOutput").ap()

    groups = [[0, 1, 2, 3], [4, 5, 6, 7]]

    with tile.TileContext(nc) as tc:
        with (
            tc.tile_pool(name="pw", bufs=1) as pw,
            tc.tile_pool(name="pstate", bufs=1) as pstate,
            tc.tile_pool(name="px", bufs=4) as px,
            tc.tile_pool(name="ptmp", bufs=18) as ptmp,
            tc.tile_pool(name="pout", bufs=3) as pout,
            tc.tile_pool(name="pps", bufs=8, space="PSUM") as pps,
            tc.tile_pool(name="pcc", bufs=4, space="DRAM") as pcc,
        ):
            wh_sb = pw.tile([128, 9, 512], FMM, tag="wh", name="wh_sb")
            wxp_sb = pw.tile([128, 3, 512], FMM, tag="wxp", name="wxp_sb")
            wxc_sb = pw.tile([128, 512], FMM, tag="wxc", name="wxc_sb")
            wx2_sb = pw.tile([128, 512], FMM, tag="wx2", name="wx2_sb")
            wo_sb = pw.tile([128, 9, 64], FMM, tag="wo", name="wo_sb")
            bg_sb = pw.tile([128, 4], F32, tag="bg", name="bg_sb")
            bo_sb = pw.tile([128, 1], F32, tag="bo", name="bo_sb")
            hmask_sb = pw.tile([128, 2 * NSLAB, WP], FMM, tag="hmask", name="hmask_sb")
            # Warm the PE clock (HAM un-throttles after ~3.4us of activity)
            # with dummy matmuls on a zeroed tile while the weight DMAs are
            # still in flight — the first real matmuls then run at 2.4 GHz.
            warm = pw.tile([128, 640], FMM, tag="warm", name="warm")
            nc.vector.memset(warm[:], 0)
            wps = pps.tile([128, 512], F32, tag="ps", name="warm_ps")
            for k in range(24):
                nc.tensor.matmul(
                    wps[:], warm[:, 0:128], warm[:, 128:640],
                    start=(k == 0), stop=(k == 23),
                )

            # x-weights first: step 1 needs no h-weights, so its matmuls can
            # start as soon as the small x-weight tiles land
            for dx in range(3):
                nc.sync.dma_start(wxp_sb[:, dx, :], wxpd[:, dx, :])
            nc.sync.dma_start(wxc_sb[:], wxcd[:])
            nc.sync.dma_start(wx2_sb[:], wx2d[:])
            nc.sync.dma_start(bg_sb[:], bgd[:])
            nc.sync.dma_start(bo_sb[:], bod[:])
            nc.sync.dma_start(hmask_sb[:], hmaskd[:])
            nc.sync.dma_start(wo_sb[:], wod[:])
            nc.sync.dma_start(wh_sb[:], whd[:])

            h_a = pstate.tile([128, HBUF, WP], FMM, tag="ha", name="h_a")
            h_b = pstate.tile([128, HBUF, WP], FMM, tag="hb", name="h_b")
            c_sb = pstate.tile([128, slab, W], F32, tag="c", name="c_sb")
            hx = pstate.tile([128, 2 * NSLAB, WP], FMM, tag="hx", name="hx")
            nc.vector.memset(h_a[:], 0)
            nc.vector.memset(h_b[:], 0)
            h_tiles = [h_a, h_b]

            # boundary tiles first: their h rows feed the halo exchange,
            # which then overlaps the interior tiles' compute. Out-conv
            # pairs are interleaved as soon as their h rows are complete so
            # their PSUM allocations never recycle a still-draining bank.
            # out-conv pairs grouped after all gate tiles: each 128x128 <->
            # 128x64 tiling-mode switch drains the PE, so keep it to two
            # switches per step. Boundary pair (1,29) last — it waits on
            # the halo exchange.
            schedule = [
                ("g", 1), ("g", 29), ("g", 5), ("g", 9), ("g", 13),
                ("g", 17), ("g", 21), ("g", 25),
                ("p", 5, 9), ("p", 13, 17), ("p", 21, 25), ("p", 1, 29),
            ]

            for t in range(1, t_steps + 1):
                h_cur = h_tiles[(t - 1) % 2]
                h_prev = h_tiles[t % 2]

                for action in schedule:
                    if action[0] == "p":
                        _, ya, yb = action
                        po = pps.tile([128, 512], F32, tag="ps", name="po")
                        k = 0
                        for dy in range(3):
                            for dx in range(3):
                                nc.tensor.matmul(
                                    po[0:64, :],
                                    wo_sb[:, dy * 3 + dx, :],
                                    h_cur[:, ya - 1 + dy : ya + 3 + dy, dx : dx + 128],
                                    start=(k == 0), stop=(k == 8),
                                    tile_position=(0, 0),
                                )
                                nc.tensor.matmul(
                                    po[64:128, :],
                                    wo_sb[:, dy * 3 + dx, :],
                                    h_cur[:, yb - 1 + dy : yb + 3 + dy, dx : dx + 128],
                                    start=(k == 0), stop=(k == 8),
                                    tile_position=(0, 64),
                                )
                                k += 1
                        ob = pout.tile([128, 4, 128], F32, tag="ostage", name="ob")
                        nc.scalar.activation(ob[:], po[:], AF.Identity, bias=bo_sb[:, 0:1])
                        nc.sync.dma_start(
                            out[t - 1, :, ya - BASE : ya - BASE + 4, :], ob[0:64]
                        )
                        nc.sync.dma_start(
                            out[t - 1, :, yb - BASE : yb - BASE + 4, :], ob[64:128]
                        )
                        continue
                    y0 = action[1]
                    n = 4 * 128
                    xs = px.tile([128, 6, WP], FMM, tag="xs", name="xs")
                    nc.sync.dma_start(xs[:, :, :], xp[t - 1, :, y0 - 1 : y0 + 5, :])
                    # col-pair tile: x rows y0+1.. (dy=2), upper = same +1 col
                    xc = px.tile([128, 4, WP], FMM, tag="xc", name="xc")
                    nc.sync.dma_start(
                        xc[0:64, :, :], xp[t - 1, 0:64, y0 + 1 : y0 + 5, :]
                    )
                    nc.sync.dma_start(
                        xc[64:128, :, 0 : WP - 1],
                        xp[t - 1, 0:64, y0 + 1 : y0 + 5, 1:WP],
                    )

                    # coc order g,i,f,o: the g-gate PSUM (feeds the longest
                    # pointwise chain) lands first; pointwise below is
                    # emitted in matching order so the strict-FIFO ACT queue
                    # never stalls on a late PSUM.
                    psums = {}
                    for coc in (3, 0, 1, 2):
                        pt = pps.tile([128, n], F32, tag="ps", name="ps")
                        psums[coc] = pt
                        mms = []
                        # x taps: (dy0,dx)+(dy1,dx) row-packed; (2,0)+(2,1)
                        # col-packed; (2,2) solo on the shifted upper half
                        for dx in range(3):
                            mms.append((
                                wxp_sb[:, dx, coc * 128 : (coc + 1) * 128],
                                xs[:, 0:4, dx : dx + 128],
                            ))
                        mms.append((
                            wxc_sb[:, coc * 128 : (coc + 1) * 128],
                            xc[:, 0:4, 0:128],
                        ))
                        # (2,2) solo tap: lower-half weights are zero, so run
                        # it full-K — a K=64 matmul (row groups 2-3 only)
                        # breaks the LDWEIGHTS/MATMUL overlap on both sides
                        # (~+107ns each way, row-group conflict).
                        mms.append((
                            wx2_sb[:, coc * 128 : (coc + 1) * 128],
                            xs[:, 1:5, 2:130],
                        ))
                        if t > 1:
                            for dy in range(3):
                                for dx in range(3):
                                    mms.append((
                                        wh_sb[:, dy * 3 + dx, coc * 128 : (coc + 1) * 128],
                                        h_prev[:, y0 - 1 + dy : y0 + 3 + dy, dx : dx + 128],
                                    ))
                        for k, (lhsT, rhs) in enumerate(mms):
                            nc.tensor.matmul(
                                pt[:], lhsT, rhs,
                                start=(k == 0), stop=(k == len(mms) - 1),
                            )

                    pt_i, pt_f, pt_o, pt_g = (psums[c] for c in range(4))
                    cw = c_sb[:, y0 - BASE : y0 - BASE + 4, :]
                    hw = h_cur[:, y0 : y0 + 4, 1:129]

                    tg = ptmp.tile([128, n], F32, tag="tmp", name="tg")
                    nc.scalar.activation(tg[:], pt_g[:], AF.Tanh, bias=bg_sb[:, 3:4])
                    si = ptmp.tile([128, n], F32, tag="tmp", name="si")
                    nc.scalar.activation(si[:], pt_i[:], AF.Sigmoid, bias=bg_sb[:, 0:1])
                    if t == 1:
                        nc.vector.tensor_mul(cw, si[:], tg[:])
                    else:
                        pr = ptmp.tile([128, n], F32, tag="tmp", name="pr")
                        nc.vector.tensor_mul(pr[:], si[:], tg[:])
                        sf = ptmp.tile([128, n], F32, tag="tmp", name="sf")
                        nc.scalar.activation(sf[:], pt_f[:], AF.Sigmoid, bias=bg_sb[:, 1:2])
                        nc.vector.tensor_mul(cw, cw, sf[:])
                        nc.vector.tensor_add(cw, cw, pr[:])
                    tct = ptmp.tile([128, n], F32, tag="tmp", name="tct")
                    nc.scalar.activation(tct[:], cw, AF.Tanh)
                    so = ptmp.tile([128, n], F32, tag="tmp", name="so")
                    nc.scalar.activation(so[:], pt_o[:], AF.Sigmoid, bias=bg_sb[:, 2:3])
                    nc.vector.tensor_mul(hw, so[:], tct[:])

                    if y0 == BASE + slab - 4:
                        # both boundary tiles done: route my top/bottom rows
                        # into neighbor shards and kick the exchange
                        cc_in = pcc.tile([2 * NSLAB, 128, WP], FMM, tag="cci", name="cc_in")
                        cc_out = pcc.tile([2, 128, WP], FMM, tag="cco", name="cc_out")
                        for j in range(NSLAB):
                            nc.vector.tensor_mul(
                                hx[:, 2 * j, :],
                                h_cur[:, BASE + slab - 1, :],
                                hmask_sb[:, 2 * j, :],
                            )
                            nc.vector.tensor_mul(
                                hx[:, 2 * j + 1, :],
                                h_cur[:, BASE, :],
                                hmask_sb[:, 2 * j + 1, :],
                            )
                        for k in range(2 * NSLAB):
                            nc.sync.dma_start(cc_in[k, :, :], hx[:, k, :])
                        nc.gpsimd.collective_compute(
                            "ReduceScatter",
                            mybir.AluOpType.add,
                            replica_groups=groups,
                            ins=[cc_in.opt()],
                            outs=[cc_out.opt()],
                        )
                        nc.sync.dma_start(h_cur[:, 0:1, :], cc_out[0, :, :])
                        nc.sync.dma_start(h_cur[:, HBUF - 1 : HBUF, :], cc_out[1, :, :])



    nc.compile()
    return nc


def prep_weights(W_gates, b_gates, W_out, b_out):
    wg = np.ascontiguousarray(W_gates, dtype=np.float32)  # [512, 192, 3, 3]
    wh = np.ascontiguousarray(
        wg[:, CIN:, :, :].reshape(512, 128, 9).transpose(1, 2, 0)
    )  # [128, 9, 512]
    wxp = np.ascontiguousarray(
        np.concatenate(
            [wg[:, :CIN, 0, :].transpose(1, 2, 0), wg[:, :CIN, 1, :].transpose(1, 2, 0)],
            axis=0,
        )
    )  # [128, 3, 512]
    wxc = np.ascontiguousarray(
        np.concatenate(
            [wg[:, :CIN, 2, 0].transpose(1, 0), wg[:, :CIN, 2, 1].transpose(1, 0)],
            axis=0,
        )
    )  # [128, 512]
    wx2 = np.zeros((128, 512), np.float32)
    wx2[64:] = wg[:, :CIN, 2, 2].transpose(1, 0)
    wo = np.ascontiguousarray(
        np.asarray(W_out, np.float32).reshape(64, 128, 9).transpose(1, 2, 0)
    )  # [128, 9, 64]
    bg = np.ascontiguousarray(np.asarray(b_gates, np.float32).reshape(4, 128).T)
    # duplicated across both column-tile halves of the paired out conv
    bo = np.ascontiguousarray(np.tile(np.asarray(b_out, np.float32).reshape(64, 1), (2, 1)))
    return {"wh": wh, "wxp": wxp, "wxc": wxc, "wx2": wx2, "wo": wo, "bg": bg, "bo": bo}


def prep_hmask(s):
    """Halo-routing mask for slab rank s: slot 2j gets my bottom row iff
    rank j is directly below me (j == s+1); slot 2j+1 gets my top row iff
    rank j is directly above me (j == s-1). Image-edge ranks contribute
    nothing, so edge halos ReduceScatter to zero == SAME padding."""
    m = np.zeros((128, 2 * NSLAB, WP), np.float32)
    for j in range(NSLAB):
        if j == s + 1:
            m[:, 2 * j, :] = 1.0
        if j == s - 1:
            m[:, 2 * j + 1, :] = 1.0
    return m


def prep_x(x, t_steps=T, slab=SLAB, h_img=H_FULL):
    """x: [T, B, C, H, W] -> list of per-core packed [T, 128, XROWS, WP].

    Core c = b * NSLAB + s covers global rows [slab*s, slab*s + slab).
    Partitions 0:64 hold x rows as-is starting at global row r0-1, 64:128
    the same rows shifted +1, so conv taps dy=0/1 share one matmul and dy=2
    reads the shifted half.
    """
    nslab = h_img // slab
    x = np.asarray(x, np.float32)
    tt, nb = x.shape[0], x.shape[1]
    cores = []
    for b in range(nb):
        xpad = np.zeros((tt, CIN, h_img + 3, WP), np.float32)
        xpad[:, :, 1 : 1 + h_img, 1 : 1 + W] = x[:, b]
        for s in range(nslab):
            r0 = slab * s
            lower = xpad[:, :, r0 : r0 + XROWS, :]
            upper = xpad[:, :, r0 + 1 : r0 + 1 + XROWS, :]
            cores.append(np.ascontiguousarray(np.concatenate([lower, upper], axis=1)))
    return cores


_NC_CACHE = {}


def _get_nc():
    key = (T, SLAB, FAST_DTYPE)
    if key not in _NC_CACHE:
        _NC_CACHE[key] = build_nc(T, SLAB)
    return _NC_CACHE[key]


def kernel(x, W_gates, b_gates, W_out, b_out):
    _import_concourse()
    from concourse import mybir
    from concourse.bass_utils import run_bass_kernel_spmd

    nc = _get_nc()
    np_mm = np.dtype(mybir.dt.np(getattr(mybir.dt, FAST_DTYPE)))
    wmap = prep_weights(W_gates, b_gates, W_out, b_out)
    xcores = prep_x(x)
    # Cast everything declared with the matmul dtype (all but bg/bo).
    wmap = {
        k: (v if k in ("bg", "bo") else np.ascontiguousarray(v.astype(np_mm)))
        for k, v in wmap.items()
    }
    in_maps = []
    for c, xc in enumerate(xcores):
        s = c % NSLAB
        in_maps.append(dict(
            wmap,
            xp=np.ascontiguousarray(xc.astype(np_mm)),
            hmask=np.ascontiguousarray(prep_hmask(s).astype(np_mm)),
        ))

    trace = bool(os.environ.get("KERNEL_TRACE"))
    kwargs = {}
    if trace:
        kwargs = {"trace": True, "tmpdir": os.environ.get("KERNEL_TRACE_DIR") or None}
    res = run_bass_kernel_spmd(nc, in_maps, core_ids=list(range(N_CORES)), **kwargs)
    if trace:
        global LAST_EXEC_NS
        LAST_EXEC_NS = res.exec_time_ns
        print(f"HW exec time: {res.exec_time_ns} ns")

    out = np.empty((T, NB, CIN, H_FULL, W), np.float32)
    for c in range(N_CORES):
        b, s = divmod(c, NSLAB)
        out[:, b, :, SLAB * s : SLAB * (s + 1), :] = res.results[c]["out"]
    return out


# revision 20
# speedup vs baseline: 1.0065x; 1.0065x over previous
"""ConvLSTM (nn_BottomConvLSTM) Trainium2 Bass kernel.

Problem (hardcoded):
  x:       [T=12, B=2, C=64, H=128, W=128] f32
  W_gates: [512, 192, 3, 3] f32,  b_gates: [512] f32
  W_out:   [64, 128, 3, 3] f32,   b_out:   [64] f32
  out:     [T, B, 64, H, W] f32

Sharding: 8 cores = B(2) x H-slabs(4 x 32 rows). Each core computes exactly
its 32-row slab every step; the 1-row h halos needed by the 3x3 convs come
from a per-step ReduceScatter across the 4 slab cores of each batch image:
every core multiplies its top/bottom h rows by a per-core 0/1 mask laid out
in destination-rank shard order, so the RS delivers to rank s exactly the
two neighbor rows it needs (and zeros at the image edges, reproducing SAME
padding). The exchange is kicked right after the two boundary row-tiles of
a step and lands while the six interior tiles compute, so it stays off the
PE critical path.

Convs run as shifted matmuls accumulating in PSUM. Matmul inputs are bf16
(PSUM accumulates fp32): bf16 enables the PE's Fast Weight Load path --
fp32r runs the array in fp32-HIGH mode, which disables FWL and leaves
~180ns of LDWEIGHTS per matmul partially unhidden. The x channels (64) are
packed twice along partitions with a +1 row shift so taps (dy=0,dx)+
(dy=1,dx) fuse into one K=128 matmul; (2,0)+(2,1) fuse via a +1
column-shifted packed copy; (2,2) runs against zero-padded full-K weights
(a K=64 matmul occupies only row groups 2-3 and breaks LDWEIGHTS/MATMUL
overlap ~+107ns on both sides) -> 5 matmuls instead of 9 for the x taps.
h taps are 9 full K=128 matmuls. LSTM pointwise runs on ACT (sigmoid/tanh)
+ DVE (mul/add). The output conv (M=64) pairs two row-groups into the two
column tiles of the PE's 128x64 mode (T0 -> PSUM partitions 0:64, T1 ->
64:128) so both stream concurrently; pairs are grouped after the gate
tiles (each tiling-mode switch drains the PE) with the boundary pair last
to wait on the halo exchange off the PE critical path.

Measured: ~1.49 ms (baseline 2.12 ms), rel err 3.9e-3 (tol 2e-2). The PE
runs power-throttled at 81.25% util (gpio_2, ~1.95 GHz effective); the
matmul stream is within ~2% of that throttled roofline, so further gains
need fewer PE columns (only fp8-DoubleRow would do that, and its ~3-5%
error exceeds tolerance).
"""

import os
import sys

import numpy as np

T = 12
CIN = 64
HID = 128
H_FULL = 128
W = 128
NB = 2
NSLAB = 4
SLAB = H_FULL // NSLAB  # 32
WP = W + 2  # zero-padded width
BASE = 1  # first slab row inside the h buffer
HBUF = SLAB + 2  # 1 halo row above + 32 slab rows + 1 halo row below
XROWS = HBUF  # x rows staged per step (same window as h)

N_CORES = 8
LAST_EXEC_NS = None

# Matmul input dtype. bf16 enables Fast Weight Load; PSUM stays fp32.
FAST_DTYPE = os.environ.get("KERNEL_MM_DTYPE", "bfloat16")


def _import_concourse():
    try:
        import concourse.bass  # noqa: F401
        return
    except ImportError:
        pass
    for p in ("/opt/trn_rl_repo", "/root/.axon_site/_ro/trn_rl_repo"):
        if os.path.isdir(p) and p not in sys.path:
            sys.path.insert(0, p)
    import concourse.bass  # noqa: F401


def build_nc(t_steps=T, slab=SLAB):
    _import_concourse()
    import concourse.tile as tile
    from concourse import bacc, mybir

    F32 = mybir.dt.float32
    FMM = getattr(mybir.dt, FAST_DTYPE)
    AF = mybir.ActivationFunctionType

    nc = bacc.Bacc("TRN2", target_bir_lowering=False, debug=False)
    xp = nc.dram_tensor("xp", [t_steps, 128, XROWS, WP], FMM, kind="ExternalInput").ap()
    whd = nc.dram_tensor("wh", [128, 9, 512], FMM, kind="ExternalInput").ap()
    wxpd = nc.dram_tensor("wxp", [128, 3, 512], FMM, kind="ExternalInput").ap()
    wxcd = nc.dram_tensor("wxc", [128, 512], FMM, kind="ExternalInput").ap()
    wx2d = nc.dram_tensor("wx2", [128, 512], FMM, kind="ExternalInput").ap()
    wod = nc.dram_tensor("wo", [128, 9, 64], FMM, kind="ExternalInput").ap()
    bgd = nc.dram_tensor("bg", [128, 4], F32, kind="ExternalInput").ap()
    bod = nc.dram_tensor("bo", [128, 1], F32, kind="ExternalInput").ap()
    # Per-core halo-routing masks, shard-major: slot 2j = "top halo for
    # rank j" (gets my bottom row iff j == my_rank+1), slot 2j+1 = "bottom
    # halo for rank j" (gets my top row iff j == my_rank-1).
    hmaskd = nc.dram_tensor("hmask", [128, 2 * NSLAB, WP], FMM, kind="ExternalInput").ap()
    out = nc.dram_tensor("out", [t_steps, 64, slab, W], F32, kind="ExternalOutput").ap()

    groups = [[0, 1, 2, 3], [4, 5, 6, 7]]

    with tile.TileContext(nc) as tc:
        with (
            tc.tile_pool(name="pw", bufs=1) as pw,
            tc.tile_pool(name="pstate", bufs=1) as pstate,
            tc.tile_pool(name="px", bufs=4) as px,
            tc.tile_pool(name="ptmp", bufs=18) as ptmp,
            tc.tile_pool(name="pout", bufs=3) as pout,
            tc.tile_pool(name="pps", bufs=8, space="PSUM") as pps,
            tc.tile_pool(name="pcc", bufs=4, space="DRAM") as pcc,
        ):
            wh_sb = pw.tile([128, 9, 512], FMM, tag="wh", name="wh_sb")
            wxp_sb = pw.tile([128, 3, 512], FMM, tag="wxp", name="wxp_sb")
            wxc_sb = pw.tile([128, 512], FMM, tag="wxc", name="wxc_sb")
            wx2_sb = pw.tile([128, 512], FMM, tag="wx2", name="wx2_sb")
            wo_sb = pw.tile([128, 9, 64], FMM, tag="wo", name="wo_sb")
            bg_sb = pw.tile([128, 4], F32, tag="bg", name="bg_sb")
            bo_sb = pw.tile([128, 1], F32, tag="bo", name="bo_sb")
            hmask_sb = pw.tile([128, 2 * NSLAB, WP], FMM, tag="hmask", name="hmask_sb")
            # Warm the PE clock (HAM un-throttles after ~3.4us of activity)
            # with dummy matmuls on a zeroed tile while the weight DMAs are
            # still in flight — the first real matmuls then run at 2.4 GHz.
            warm = pw.tile([128, 640], FMM, tag="warm", name="warm")
            nc.vector.memset(warm[:], 0)
            wps = pps.tile([128, 512], F32, tag="ps", name="warm_ps")
            for k in range(12):
                nc.tensor.matmul(
                    wps[:], warm[:, 0:128], warm[:, 128:640],
                    start=(k == 0), stop=(k == 11),
                )

            # x-weights first: step 1 needs no h-weights, so its matmuls can
            # start as soon as the small x-weight tiles land
            for dx in range(3):
                nc.sync.dma_start(wxp_sb[:, dx, :], wxpd[:, dx, :])
            nc.sync.dma_start(wxc_sb[:], wxcd[:])
            nc.sync.dma_start(wx2_sb[:], wx2d[:])
            nc.sync.dma_start(bg_sb[:], bgd[:])
            nc.sync.dma_start(bo_sb[:], bod[:])
            nc.sync.dma_start(hmask_sb[:], hmaskd[:])
            nc.sync.dma_start(wo_sb[:], wod[:])
            nc.sync.dma_start(wh_sb[:], whd[:])

            h_a = pstate.tile([128, HBUF, WP], FMM, tag="ha", name="h_a")
            h_b = pstate.tile([128, HBUF, WP], FMM, tag="hb", name="h_b")
            c_sb = pstate.tile([128, slab, W], F32, tag="c", name="c_sb")
            hx = pstate.tile([128, 2 * NSLAB, WP], FMM, tag="hx", name="hx")
            nc.vector.memset(h_a[:], 0)
            nc.vector.memset(h_b[:], 0)
            h_tiles = [h_a, h_b]

            # boundary tiles first: their h rows feed the halo exchange,
            # which then overlaps the interior tiles' compute. Out-conv
            # pairs are interleaved as soon as their h rows are complete so
            # their PSUM allocations never recycle a still-draining bank.
            # out-conv pairs grouped after all gate tiles: each 128x128 <->
            # 128x64 tiling-mode switch drains the PE, so keep it to two
            # switches per step. Boundary pair (1,29) last — it waits on
            # the halo exchange.
            schedule = [
                ("g", 1), ("g", 29), ("g", 5), ("g", 9), ("g", 13),
                ("g", 17), ("g", 21), ("g", 25),
                ("p", 5, 9), ("p", 13, 17), ("p", 21, 25), ("p", 1, 29),
            ]

            for t in range(1, t_steps + 1):
                h_cur = h_tiles[(t - 1) % 2]
                h_prev = h_tiles[t % 2]

                for action in schedule:
                    if action[0] == "p":
                        _, ya, yb = action
                        po = pps.tile([128, 512], F32, tag="ps", name="po")
                        k = 0
                        for dy in range(3):
                            for dx in range(3):
                                nc.tensor.matmul(
                                    po[0:64, :],
                                    wo_sb[:, dy * 3 + dx, :],
                                    h_cur[:, ya - 1 + dy : ya + 3 + dy, dx : dx + 128],
                                    start=(k == 0), stop=(k == 8),
                                    tile_position=(0, 0),
                                )
                                nc.tensor.matmul(
                                    po[64:128, :],
                                    wo_sb[:, dy * 3 + dx, :],
                                    h_cur[:, yb - 1 + dy : yb + 3 + dy, dx : dx + 128],
                                    start=(k == 0), stop=(k == 8),
                                    tile_position=(0, 64),
                                )
                                k += 1
                        ob = pout.tile([128, 4, 128], F32, tag="ostage", name="ob")
                        nc.scalar.activation(ob[:], po[:], AF.Identity, bias=bo_sb[:, 0:1])
                        nc.sync.dma_start(
                            out[t - 1, :, ya - BASE : ya - BASE + 4, :], ob[0:64]
                        )
                        nc.sync.dma_start(
                            out[t - 1, :, yb - BASE : yb - BASE + 4, :], ob[64:128]
                        )
                        continue
                    y0 = action[1]
                    n = 4 * 128
                    xs = px.tile([128, 6, WP], FMM, tag="xs", name="xs")
                    nc.sync.dma_start(xs[:, :, :], xp[t - 1, :, y0 - 1 : y0 + 5, :])
                    # col-pair tile: x rows y0+1.. (dy=2), upper = same +1 col
                    xc = px.tile([128, 4, WP], FMM, tag="xc", name="xc")
                    nc.sync.dma_start(
                        xc[0:64, :, :], xp[t - 1, 0:64, y0 + 1 : y0 + 5, :]
                    )
                    nc.sync.dma_start(
                        xc[64:128, :, 0 : WP - 1],
                        xp[t - 1, 0:64, y0 + 1 : y0 + 5, 1:WP],
                    )

                    # coc order g,i,f,o: the g-gate PSUM (feeds the longest
                    # pointwise chain) lands first; pointwise below is
                    # emitted in matching order so the strict-FIFO ACT queue
                    # never stalls on a late PSUM.
                    psums = {}
                    for coc in (3, 0, 1, 2):
                        pt = pps.tile([128, n], F32, tag="ps", name="ps")
                        psums[coc] = pt
                        mms = []
                        # x taps: (dy0,dx)+(dy1,dx) row-packed; (2,0)+(2,1)
                        # col-packed; (2,2) solo on the shifted upper half
                        for dx in range(3):
                            mms.append((
                                wxp_sb[:, dx, coc * 128 : (coc + 1) * 128],
                                xs[:, 0:4, dx : dx + 128],
                            ))
                        mms.append((
                            wxc_sb[:, coc * 128 : (coc + 1) * 128],
                            xc[:, 0:4, 0:128],
                        ))
                        # (2,2) solo tap: lower-half weights are zero, so run
                        # it full-K — a K=64 matmul (row groups 2-3 only)
                        # breaks the LDWEIGHTS/MATMUL overlap on both sides
                        # (~+107ns each way, row-group conflict).
                        mms.append((
                            wx2_sb[:, coc * 128 : (coc + 1) * 128],
                            xs[:, 1:5, 2:130],
                        ))
                        if t > 1:
                            for dy in range(3):
                                for dx in range(3):
                                    mms.append((
                                        wh_sb[:, dy * 3 + dx, coc * 128 : (coc + 1) * 128],
                                        h_prev[:, y0 - 1 + dy : y0 + 3 + dy, dx : dx + 128],
                                    ))
                        for k, (lhsT, rhs) in enumerate(mms):
                            nc.tensor.matmul(
                                pt[:], lhsT, rhs,
                                start=(k == 0), stop=(k == len(mms) - 1),
                            )

                    pt_i, pt_f, pt_o, pt_g = (psums[c] for c in range(4))
                    cw = c_sb[:, y0 - BASE : y0 - BASE + 4, :]
                    hw = h_cur[:, y0 : y0 + 4, 1:129]

                    tg = ptmp.tile([128, n], F32, tag="tmp", name="tg")
                    nc.scalar.activation(tg[:], pt_g[:], AF.Tanh, bias=bg_sb[:, 3:4])
                    si = ptmp.tile([128, n], F32, tag="tmp", name="si")
                    nc.scalar.activation(si[:], pt_i[:], AF.Sigmoid, bias=bg_sb[:, 0:1])
                    if t == 1:
                        nc.vector.tensor_mul(cw, si[:], tg[:])
                    else:
                        pr = ptmp.tile([128, n], F32, tag="tmp", name="pr")
                        nc.vector.tensor_mul(pr[:], si[:], tg[:])
                        sf = ptmp.tile([128, n], F32, tag="tmp", name="sf")
                        nc.scalar.activation(sf[:], pt_f[:], AF.Sigmoid, bias=bg_sb[:, 1:2])
                        nc.vector.tensor_mul(cw, cw, sf[:])
                        nc.vector.tensor_add(cw, cw, pr[:])
                    tct = ptmp.tile([128, n], F32, tag="tmp", name="tct")
                    nc.scalar.activation(tct[:], cw, AF.Tanh)
                    so = ptmp.tile([128, n], F32, tag="tmp", name="so")
                    nc.scalar.activation(so[:], pt_o[:], AF.Sigmoid, bias=bg_sb[:, 2:3])
                    nc.vector.tensor_mul(hw, so[:], tct[:])

                    if y0 == BASE + slab - 4:
                        # both boundary tiles done: route my top/bottom rows
                        # into neighbor shards and kick the exchange
                        cc_in = pcc.tile([2 * NSLAB, 128, WP], FMM, tag="cci", name="cc_in")
                        cc_out = pcc.tile([2, 128, WP], FMM, tag="cco", name="cc_out")
                        for j in range(NSLAB):
                            nc.vector.tensor_mul(
                                hx[:, 2 * j, :],
                                h_cur[:, BASE + slab - 1, :],
                                hmask_sb[:, 2 * j, :],
                            )
                            nc.vector.tensor_mul(
                                hx[:, 2 * j + 1, :],
                                h_cur[:, BASE, :],
                                hmask_sb[:, 2 * j + 1, :],
                            )
                        for k in range(2 * NSLAB):
                            nc.sync.dma_start(cc_in[k, :, :], hx[:, k, :])
                        nc.gpsimd.collective_compute(
                            "ReduceScatter",
                            mybir.AluOpType.add,
                            replica_groups=groups,
                            ins=[cc_in.opt()],
                            outs=[cc_out.opt()],
                        )
                        nc.sync.dma_start(h_cur[:, 0:1, :], cc_out[0, :, :])
                        nc.sync.dma_start(h_cur[:, HBUF - 1 : HBUF, :], cc_out[1, :, :])



    nc.compile()
    return nc


def prep_weights(W_gates, b_gates, W_out, b_out):
    wg = np.ascontiguousarray(W_gates, dtype=np.float32)  # [512, 192, 3, 3]
    wh = np.ascontiguousarray(
        wg[:, CIN:, :, :].reshape(512, 128, 9).transpose(1, 2, 0)
    )  # [128, 9, 512]
    wxp = np.ascontiguousarray(
        np.concatenate(
            [wg[:, :CIN, 0, :].transpose(1, 2, 0), wg[:, :CIN, 1, :].transpose(1, 2, 0)],
            axis=0,
        )
    )  # [128, 3, 512]
    wxc = np.ascontiguousarray(
        np.concatenate(
            [wg[:, :CIN, 2, 0].transpose(1, 0), wg[:, :CIN, 2, 1].transpose(1, 0)],
            axis=0,
        )
    )  # [128, 512]
    wx2 = np.zeros((128, 512), np.float32)
    wx2[64:] = wg[:, :CIN, 2, 2].transpose(1, 0)
    wo = np.ascontiguousarray(
        np.asarray(W_out, np.float32).reshape(64, 128, 9).transpose(1, 2, 0)
    )  # [128, 9, 64]
    bg = np.ascontiguousarray(np.asarray(b_gates, np.float32).reshape(4, 128).T)
    # duplicated across both column-tile halves of the paired out conv
    bo = np.ascontiguousarray(np.tile(np.asarray(b_out, np.float32).reshape(64, 1), (2, 1)))
    return {"wh": wh, "wxp": wxp, "wxc": wxc, "wx2": wx2, "wo": wo, "bg": bg, "bo": bo}


def prep_hmask(s):
    """Halo-routing mask for slab rank s: slot 2j gets my bottom row iff
    rank j is directly below me (j == s+1); slot 2j+1 gets my top row iff
    rank j is directly above me (j == s-1). Image-edge ranks contribute
    nothing, so edge halos ReduceScatter to zero == SAME padding."""
    m = np.zeros((128, 2 * NSLAB, WP), np.float32)
    for j in range(NSLAB):
        if j == s + 1:
            m[:, 2 * j, :] = 1.0
        if j == s - 1:
            m[:, 2 * j + 1, :] = 1.0
    return m


def prep_x(x, t_steps=T, slab=SLAB, h_img=H_FULL):
    """x: [T, B, C, H, W] -> list of per-core packed [T, 128, XROWS, WP].

    Core c = b * NSLAB + s covers global rows [slab*s, slab*s + slab).
    Partitions 0:64 hold x rows as-is starting at global row r0-1, 64:128
    the same rows shifted +1, so conv taps dy=0/1 share one matmul and dy=2
    reads the shifted half.
    """
    nslab = h_img // slab
    x = np.asarray(x, np.float32)
    tt, nb = x.shape[0], x.shape[1]
    cores = []
    for b in range(nb):
        xpad = np.zeros((tt, CIN, h_img + 3, WP), np.float32)
        xpad[:, :, 1 : 1 + h_img, 1 : 1 + W] = x[:, b]
        for s in range(nslab):
            r0 = slab * s
            lower = xpad[:, :, r0 : r0 + XROWS, :]
            upper = xpad[:, :, r0 + 1 : r0 + 1 + XROWS, :]
            cores.append(np.ascontiguousarray(np.concatenate([lower, upper], axis=1)))
    return cores


_NC_CACHE = {}


def _get_nc():
    key = (T, SLAB, FAST_DTYPE)
    if key not in _NC_CACHE:
        _NC_CACHE[key] = build_nc(T, SLAB)
    return _NC_CACHE[key]


def kernel(x, W_gates, b_gates, W_out, b_out):
    _import_concourse()
    from concourse import mybir
    from concourse.bass_utils import run_bass_kernel_spmd

    nc = _get_nc()
    np_mm = np.dtype(mybir.dt.np(getattr(mybir.dt, FAST_DTYPE)))
    wmap = prep_weights(W_gates, b_gates, W_out, b_out)
    xcores = prep_x(x)
    # Cast everything declared with the matmul dtype (all but bg/bo).
    wmap = {
        k: (v if k in ("bg", "bo") else np.ascontiguousarray(v.astype(np_mm)))
        for k, v in wmap.items()
    }
    in_maps = []
    for c, xc in enumerate(xcores):
        s = c % NSLAB
        in_maps.append(dict(
            wmap,
            xp=np.ascontiguousarray(xc.astype(np_mm)),
            hmask=np.ascontiguousarray(prep_hmask(s).astype(np_mm)),
        ))

    trace = bool(os.environ.get("KERNEL_TRACE"))
    kwargs = {}
    if trace:
        kwargs = {"trace": True, "tmpdir": os.environ.get("KERNEL_TRACE_DIR") or None}
    res = run_bass_kernel_spmd(nc, in_maps, core_ids=list(range(N_CORES)), **kwargs)
    if trace:
        global LAST_EXEC_NS
        LAST_EXEC_NS = res.exec_time_ns
        print(f"HW exec time: {res.exec_time_ns} ns")

    out = np.empty((T, NB, CIN, H_FULL, W), np.float32)
    for c in range(N_CORES):
        b, s = divmod(c, NSLAB)
        out[:, b, :, SLAB * s : SLAB * (s + 1), :] = res.results[c]["out"]
    return out
